# revision 8
# baseline (speedup 1.0000x reference)
"""Single-head attention block (B=8, N=2048, D=768) on 8 Trainium2 NeuronCores.

Strategy: pure data-parallel over the batch dimension — one batch element per
NeuronCore. Each core computes, for its x_b [N, D]:

  q = x@Wq + bq, k = x@Wk + bk, v = x@Wv        (fp8 hi/lo matmuls, fp32 PSUM)
  expT[j, i] = exp((q_i . k_j) / sqrt(D))        (no max-subtraction: scaled
                                                  scores are bounded ~|2.2|)
  outU[i, :] = sum_j expT[j, i] * v_aug[j, :]    (v_aug has a const column ->
                                                  col D holds the softmax row
                                                  sums)
  out[i, e] = outU[i, e] / outU[i, D] + bv[e]    (v-bias folded to the end:
                                                  softmax rows sum to 1)

All big matmuls run on the fp8-e4m3 DoubleRow path (2 contraction chunks per
instruction, ~2x the fp16 column rate on TRN2):

  - projections: 3-product hi/lo split. A = A_hi + A_lo (each e4m3), the
    matmul computes A_hi*W_hi + A_lo*W_hi + A_hi*W_lo (3 DoubleRow instrs per
    2 chunks = 0.75x the fp16 cycle cost) which carries ~fp16 accuracy.
    Weights are pre-scaled by SW=32 host-side so their hi/lo residuals stay
    above e4m3's minimum subnormal (2^-9); the scale is folded back in the
    exp activation scale and the softmax-sum column.
  - scores: pure fp8 (q, k quantized to e4m3, 0.5x cycle cost). The score
    quantization passes through softmax's shift-invariance mostly harmlessly:
    measured whole-output rel err 1.05e-2 vs the 2e-2 budget.
  - AV: 3-product hi/lo split of both exp and v (0.75x, ~fp16 accuracy).

Layouts keep the contraction axis on SBUF partitions; no on-chip transposes:
  - qT, kT [D_part, N_free]  fp8, scaled by SW (projections computed
    transposed); exp scale absorbs 1/SW^2
  - scoresT computed transposed: scoresT[j_part, i_free]
  - v natural [N_part, D_free] hi/lo fp8, scaled by SW; the ones-column is SW
    so the row-sum column matches the SW-scaled numerator
  - exp stored hi/lo fp8 [N_part(j), N_free(i)]
"""

import math
import sys

import numpy as np

sys.path.insert(0, "/opt/trn_rl_repo")

import ml_dtypes  # noqa: E402

import concourse.bass as bass  # noqa: E402
import concourse.tile as tile  # noqa: E402
from concourse import bacc, mybir  # noqa: E402
from concourse import bass_utils  # noqa: E402

B, N, D = 8, 2048, 768
P = 128
DC = D // P  # 6 chunks of the embedding/contraction dim
PC = DC // 2  # 3 DoubleRow pair-chunks
NT = N // P  # 16 chunks of the sequence dim
NP2 = NT // 2  # 8 DoubleRow pair-chunks of the sequence dim
FD = 512  # matmul free-dim tile (one fp32 PSUM bank; ISA caps PSUM writes)
CDT = mybir.dt.float16
CDT_NP = np.float16
F8 = mybir.dt.float8e4
F8_NP = ml_dtypes.float8_e4m3
DR = mybir.MatmulPerfMode.DoubleRow
F32 = mybir.dt.float32
SW = 32.0  # host-side weight pre-scale (keeps W hi/lo splits in normal range)
INV_SQRT_D = 1.0 / math.sqrt(D)

# Filled by kernel() so a test harness can report the profiled HW time.
LAST_RESULT = None

# PSUM pool granularity: (tile_cols, bufs).
PSUM_GRAN = (1024, 4)


def _emit(tc, out, xh, xl, wqh, wql, wkh, wkl, wvh, wvl, bqk, bv, repeat=1):
    nc = tc.nc
    Ident = mybir.ActivationFunctionType.Identity
    Copy = mybir.ActivationFunctionType.Copy
    Exp = mybir.ActivationFunctionType.Exp
    Mult = mybir.AluOpType.mult
    Sub = mybir.AluOpType.subtract
    Add = mybir.AluOpType.add

    with (
        tc.tile_pool(name="const", bufs=1) as const,
        tc.tile_pool(name="data", bufs=1) as data,
        tc.tile_pool(name="expp", bufs=1) as expp,
        tc.tile_pool(name="tmpp", bufs=3) as tmpp,
        tc.tile_pool(name="psum", bufs=PSUM_GRAN[1], space="PSUM") as psum,
        tc.tile_pool(name="outp", bufs=3) as outp,
        tc.tile_pool(name="small", bufs=4) as small,
    ):
        # Persistent activations (all fp8; q/k/v carry the SW scale)
        qT = data.tile([P, DC, N], F8)  # qT[p, o, n] = SW*q[n, o*128+p]
        kT = data.tile([P, DC, N], F8)
        vh = data.tile([P, NT, D + 16], F8)  # v hi; col D = SW
        vl = data.tile([P, NT, D + 16], F8)  # v lo; col D = 0
        eh = expp.tile([P, NT, N], F8)  # exp hi: eh[p,t,i]~exp(s[i,t*128+p])
        el = expp.tile([P, NT, N], F8)  # exp lo (residual)
        xhs = data.tile([P, DC, N], F8)  # x hi, transposed: [d-part, n]
        xls = data.tile([P, DC, N], F8)  # x lo
        # wq is host-packed ec-major: wq*[p, ec, dc, c] = Wq'[dc*128+p,
        # ec*128+c] so the ec=0 stationary slices land in one small first DMA.
        wqhs = data.tile([P, DC, DC, P], F8)
        wqls = data.tile([P, DC, DC, P], F8)
        wkhs = data.tile([P, DC, D], F8)  # wk*[p, o, e] = Wk'[o*128+p, e]
        wkls = data.tile([P, DC, D], F8)
        wvhs = data.tile([P, DC, D], F8)
        wvls = data.tile([P, DC, D], F8)
        bqks = const.tile([P, 2 * DC], F32)  # host-packed: [p, o] = SW*bq,
        bqs = bqks[:, 0:DC]  # [p, DC+o] = SW*bk
        bks = bqks[:, DC : 2 * DC]
        bvb = const.tile([P, D], F32)  # bvb[p, e] = bv[e] (partition-bcast)

        def body():
            # Input loads in dependency order: the first projection needs wq +
            # x first; bvb is only read by the final epilogue.
            nc.sync.dma_start(wqhs[:, 0], wqh[0])
            nc.scalar.dma_start(wqls[:, 0], wql[0])
            nc.scalar.dma_start(bqks[:], bqk[:])
            H = N // 2
            for dc in range(DC):
                nc.sync.dma_start(
                    xhs[:, dc, 0:H],
                    xh[dc * P : (dc + 1) * P, 0:H].rearrange("(o p) n -> p o n", p=P),
                )
                nc.scalar.dma_start(
                    xls[:, dc, 0:H],
                    xl[dc * P : (dc + 1) * P, 0:H].rearrange("(o p) n -> p o n", p=P),
                )
                if dc < DC - 1:
                    nc.sync.dma_start(wqhs[:, dc + 1], wqh[dc + 1])
                    nc.scalar.dma_start(wqls[:, dc + 1], wql[dc + 1])
            for dc in range(DC):
                nc.sync.dma_start(
                    xhs[:, dc, H:N],
                    xh[dc * P : (dc + 1) * P, H:N].rearrange("(o p) n -> p o n", p=P),
                )
                nc.scalar.dma_start(
                    xls[:, dc, H:N],
                    xl[dc * P : (dc + 1) * P, H:N].rearrange("(o p) n -> p o n", p=P),
                )
            nc.sync.dma_start(wkhs[:], wkh.rearrange("(o p) e -> p o e", p=P))
            nc.scalar.dma_start(wkls[:], wkl.rearrange("(o p) e -> p o e", p=P))
            nc.sync.dma_start(wvhs[:], wvh.rearrange("(o p) e -> p o e", p=P))
            nc.scalar.dma_start(wvls[:], wvl.rearrange("(o p) e -> p o e", p=P))
            nc.scalar.dma_start(
                bvb[:],
                bass.AP(tensor=bv.tensor, offset=bv.offset, ap=[[0, P], *bv.ap]),
            )

            def mm3(ps, hi_pair_lhs, lo_pair_lhs, hi_pair_rhs, lo_pair_rhs,
                    first, last):
                """One 3-product hi/lo DoubleRow pair-chunk contraction:
                hi.hi + lo.hi + hi.lo accumulated into ps."""
                nc.tensor.matmul(ps, lhsT=hi_pair_lhs, rhs=hi_pair_rhs,
                                 start=first, stop=False, perf_mode=DR)
                nc.tensor.matmul(ps, lhsT=lo_pair_lhs, rhs=hi_pair_rhs,
                                 start=False, stop=False, perf_mode=DR)
                nc.tensor.matmul(ps, lhsT=hi_pair_lhs, rhs=lo_pair_rhs,
                                 start=False, stop=last, perf_mode=DR)

            # q, k projections in transposed layout:
            # qT[e, n] = SW*(sum_d Wq[d, e] * xT[d, n] + bq[e]) (e on parts)
            # q runs n-block-outermost so its first matmuls depend only on
            # wq[ec0] + the first half of x.
            G = PSUM_GRAN[0]
            for nb in range((N + G - 1) // G):
                for ec in range(DC):
                    ps = psum.tile([P, G], F32, tag="ps", name="ps")
                    for pc in range(PC):
                        d2 = slice(2 * pc, 2 * pc + 2)
                        for h in range(G // FD):
                            col = nb * G + h * FD
                            mm3(
                                ps[:, h * FD : (h + 1) * FD],
                                wqhs[:, ec, d2, :],
                                wqls[:, ec, d2, :],
                                xhs[:, d2, col : col + FD],
                                xls[:, d2, col : col + FD],
                                first=(pc == 0),
                                last=(pc == PC - 1),
                            )
                    nc.scalar.activation(
                        qT[:, ec, nb * G : (nb + 1) * G],
                        ps[:],
                        Ident,
                        bias=bqs[:, ec : ec + 1],
                    )
            for ec in range(DC):
                pss = [
                    psum.tile([P, G], F32, tag="ps", name="ps")
                    for _ in range(N // G)
                ]
                for pc in range(PC):
                    d2 = slice(2 * pc, 2 * pc + 2)
                    e2 = slice(ec * P, (ec + 1) * P)
                    for nj in range(N // FD):
                        ps = pss[nj // (G // FD)]
                        col = (nj % (G // FD)) * FD
                        mm3(
                            ps[:, col : col + FD],
                            wkhs[:, d2, e2],
                            wkls[:, d2, e2],
                            xhs[:, d2, nj * FD : (nj + 1) * FD],
                            xls[:, d2, nj * FD : (nj + 1) * FD],
                            first=(pc == 0),
                            last=(pc == PC - 1),
                        )
                for g, ps in enumerate(pss):
                    nc.scalar.activation(
                        kT[:, ec, g * G : (g + 1) * G],
                        ps[:],
                        Ident,
                        bias=bks[:, ec : ec + 1],
                    )

            # v projection in natural layout: v[n, e] = SW * sum_d x[n, d] *
            # Wv[d, e] (bias deferred to the epilogue). hi part from ACT,
            # lo residual from DVE. Column D gets SW (hi) / 0 (lo) so the AV
            # matmul also produces SW-scaled softmax row sums.
            for nt in range(NT):
                ps = psum.tile([P, PSUM_GRAN[0]], F32, tag="ps", name="ps")
                n2 = slice(nt * P, (nt + 1) * P)
                for pc in range(PC):
                    d2 = slice(2 * pc, 2 * pc + 2)
                    mm3(
                        ps[:, 0:FD],
                        xhs[:, d2, n2], xls[:, d2, n2],
                        wvhs[:, d2, 0:FD], wvls[:, d2, 0:FD],
                        first=(pc == 0), last=(pc == PC - 1),
                    )
                    mm3(
                        ps[:, FD:D],
                        xhs[:, d2, n2], xls[:, d2, n2],
                        wvhs[:, d2, FD:D], wvls[:, d2, FD:D],
                        first=(pc == 0), last=(pc == PC - 1),
                    )
                nc.scalar.activation(vh[:, nt, 0:D], ps[:, 0:D], Copy)
                nc.vector.scalar_tensor_tensor(
                    vl[:, nt, 0:D], ps[:, 0:D], 1.0, vh[:, nt, 0:D],
                    op0=Mult, op1=Sub,
                )
                nc.vector.memset(vh[:, nt, D : D + 1], SW)
                nc.vector.memset(vl[:, nt, D : D + 1], 0.0)

            # scoresT[j, i] = sum_d kT[d, j] * qT[d, i] (pure fp8 DoubleRow);
            # exp with 1/(SW^2 sqrt(D)) folded into the activation scale.
            # exp lands in a rotating fp16 buffer, then DVE splits it into
            # fp8 hi + lo for the AV 3-product.
            for jt in range(NT):
                pss = [
                    psum.tile([P, G], F32, tag="ps", name="ps")
                    for _ in range(N // G)
                ]
                for pc in range(PC):
                    d2 = slice(2 * pc, 2 * pc + 2)
                    lhsT = kT[:, d2, jt * P : (jt + 1) * P]
                    for ni in range(N // FD):
                        ps = pss[ni // (G // FD)]
                        col = (ni % (G // FD)) * FD
                        nc.tensor.matmul(
                            ps[:, col : col + FD],
                            lhsT=lhsT,
                            rhs=qT[:, d2, ni * FD : (ni + 1) * FD],
                            start=(pc == 0),
                            stop=(pc == PC - 1),
                            perf_mode=DR,
                        )
                et = tmpp.tile([P, N], CDT, tag="et", name="et")
                for g, ps in enumerate(pss):
                    nc.scalar.activation(
                        et[:, g * G : (g + 1) * G],
                        ps[:],
                        Exp,
                        scale=INV_SQRT_D / (SW * SW),
                    )
                nc.vector.tensor_scalar_mul(eh[:, jt], et[:], 1.0)
                nc.vector.scalar_tensor_tensor(
                    el[:, jt], et[:], 1.0, eh[:, jt], op0=Mult, op1=Sub,
                )

            # out[i, e] = sum_j exp[j, i] * v[j, e] via the hi/lo 3-product;
            # col D accumulates SW * row sums (SW cancels in the division).
            for it in range(NT):
                ps = psum.tile([P, PSUM_GRAN[0]], F32, tag="ps", name="ps")
                i2 = slice(it * P, (it + 1) * P)
                for jp in range(NP2):
                    j2 = slice(2 * jp, 2 * jp + 2)
                    mm3(
                        ps[:, 0:FD],
                        eh[:, j2, i2], el[:, j2, i2],
                        vh[:, j2, 0:FD], vl[:, j2, 0:FD],
                        first=(jp == 0), last=(jp == NP2 - 1),
                    )
                    mm3(
                        ps[:, FD : D + 1],
                        eh[:, j2, i2], el[:, j2, i2],
                        vh[:, j2, FD : D + 1], vl[:, j2, FD : D + 1],
                        first=(jp == 0), last=(jp == NP2 - 1),
                    )
                recip = small.tile([P, 1], F32, tag="recip", name="recip")
                nc.vector.reciprocal(recip[:], ps[:, D : D + 1])
                # fp16 store (half the output DMA); host upcasts to f32.
                of = outp.tile([P, D], CDT, tag="of", name="of")
                nc.vector.scalar_tensor_tensor(
                    of[:],
                    ps[:, 0:D],
                    recip[:],
                    bvb[:],
                    op0=Mult,
                    op1=Add,
                )
                nc.sync.dma_start(out[it * P : (it + 1) * P, :], of[:])

        if repeat == 1:
            body()
        else:
            hints = (
                mybir.EngineType.PE,
                mybir.EngineType.Activation,
                mybir.EngineType.DVE,
                mybir.EngineType.SP,
            )
            with tc.For_i(0, repeat, 1, hint_engines=hints):
                body()


def _build(repeat=1):
    nc = bacc.Bacc(
        "TRN2",
        target_bir_lowering=False,
        debug=False,
        enable_asserts=False,
        num_devices=B,
    )
    xh = nc.dram_tensor("xh", [D, N], F8, kind="ExternalInput").ap()
    xl = nc.dram_tensor("xl", [D, N], F8, kind="ExternalInput").ap()
    wqh = nc.dram_tensor("wqh", [DC, P, DC, P], F8, kind="ExternalInput").ap()
    wql = nc.dram_tensor("wql", [DC, P, DC, P], F8, kind="ExternalInput").ap()
    wkh = nc.dram_tensor("wkh", [D, D], F8, kind="ExternalInput").ap()
    wkl = nc.dram_tensor("wkl", [D, D], F8, kind="ExternalInput").ap()
    wvh = nc.dram_tensor("wvh", [D, D], F8, kind="ExternalInput").ap()
    wvl = nc.dram_tensor("wvl", [D, D], F8, kind="ExternalInput").ap()
    bqk = nc.dram_tensor("bqk", [P, 2 * DC], F32, kind="ExternalInput").ap()
    bv = nc.dram_tensor("bv", [D], F32, kind="ExternalInput").ap()
    out = nc.dram_tensor("out", [N, D], CDT, kind="ExternalOutput").ap()
    with tile.TileContext(nc) as tc:
        _emit(tc, out, xh, xl, wqh, wql, wkh, wkl, wvh, wvl, bqk, bv,
              repeat=repeat)
    nc.compile()
    return nc


def _split8(a):
    """Split fp32/64 array into e4m3 hi + lo with hi = rne(a)."""
    hi = a.astype(F8_NP)
    lo = (a - hi.astype(np.float32)).astype(F8_NP)
    return hi, lo


def make_in_maps(inputs):
    x = np.asarray(inputs["x"], np.float32)
    wq = np.asarray(inputs["Wq"], np.float32) * SW
    wk = np.asarray(inputs["Wk"], np.float32) * SW
    wv = np.asarray(inputs["Wv"], np.float32) * SW
    # ec-major packing: wq2[ec, p, dc, c] = Wq'[dc*128+p, ec*128+c]
    def qpack(w):
        return np.ascontiguousarray(
            w.reshape(DC, P, DC, P).transpose(2, 1, 0, 3))
    wqh, wql = _split8(qpack(wq))
    wkh, wkl = _split8(wk)
    wvh, wvl = _split8(wv)
    bq = np.asarray(inputs["bq"], np.float32) * SW
    bk = np.asarray(inputs["bk"], np.float32) * SW
    bv = np.ascontiguousarray(np.asarray(inputs["bv"], np.float32))
    # bqk[p, o] = SW*bq[o*128+p]; bqk[p, DC+o] = SW*bk[o*128+p]
    bqk = np.ascontiguousarray(
        np.concatenate([bq.reshape(DC, P).T, bk.reshape(DC, P).T], axis=1)
    )
    maps = []
    for b in range(B):
        xT = np.ascontiguousarray(x[b].T)
        xh, xl = _split8(xT)
        maps.append(
            {
                "xh": xh,
                "xl": xl,
                "wqh": wqh,
                "wql": wql,
                "wkh": wkh,
                "wkl": wkl,
                "wvh": wvh,
                "wvl": wvl,
                "bqk": bqk,
                "bv": bv,
            }
        )
    return maps


_NC_CACHE = {}


def kernel(**inputs):
    global LAST_RESULT
    in_maps = make_in_maps(inputs)

    if 1 not in _NC_CACHE:
        _NC_CACHE[1] = _build()
    nc = _NC_CACHE[1]
    res = None
    for attempt in range(3):
        try:
            res = bass_utils.run_bass_kernel_spmd(nc, in_maps, core_ids=list(range(B)))
            break
        except Exception:
            if attempt == 2:
                raise
    LAST_RESULT = res
    return np.stack([res.results[c]["out"] for c in range(B)], axis=0).astype(
        np.float32
    )


if __name__ == "__main__":
    rng = np.random.default_rng(0)
    demo = {
        "x": rng.standard_normal((B, N, D), dtype=np.float32),
        "Wq": rng.uniform(-0.036, 0.036, (D, D)).astype(np.float32),
        "bq": rng.uniform(-0.036, 0.036, D).astype(np.float32),
        "Wk": rng.uniform(-0.036, 0.036, (D, D)).astype(np.float32),
        "bk": rng.uniform(-0.036, 0.036, D).astype(np.float32),
        "Wv": rng.uniform(-0.036, 0.036, (D, D)).astype(np.float32),
        "bv": rng.uniform(-0.036, 0.036, D).astype(np.float32),
    }
    out = kernel(**demo)
    print("out", out.shape, out.dtype, float(np.abs(out).max()))


# revision 11
# speedup vs baseline: 1.0722x; 1.0722x over previous
"""Single-head attention block (B=8, N=2048, D=768) on 8 Trainium2 NeuronCores.

Strategy: pure data-parallel over the batch dimension — one batch element per
NeuronCore. Each core computes, for its x_b [N, D]:

  q = x@Wq + bq, k = x@Wk + bk, v = x@Wv        (fp8 hi/lo matmuls, fp32 PSUM)
  expT[j, i] = exp((q_i . k_j) / sqrt(D))        (no max-subtraction: scaled
                                                  scores are bounded ~|2.2|)
  outU[i, :] = sum_j expT[j, i] * v_aug[j, :]    (v_aug has a const column ->
                                                  col D holds the softmax row
                                                  sums)
  out[i, e] = outU[i, e] / outU[i, D] + bv[e]    (v-bias folded to the end:
                                                  softmax rows sum to 1)

All big matmuls run on the fp8-e4m3 DoubleRow path (2 contraction chunks per
instruction, ~2x the fp16 column rate on TRN2):

  - projections: 3-product hi/lo split. A = A_hi + A_lo (each e4m3), the
    matmul computes A_hi*W_hi + A_lo*W_hi + A_hi*W_lo (3 DoubleRow instrs per
    2 chunks = 0.75x the fp16 cycle cost) which carries ~fp16 accuracy.
    Weights are pre-scaled by SW=32 host-side so their hi/lo residuals stay
    above e4m3's minimum subnormal (2^-9); the scale is folded back in the
    exp activation scale and the softmax-sum column.
  - scores: pure fp8 (q, k quantized to e4m3, 0.5x cycle cost). The score
    quantization passes through softmax's shift-invariance mostly harmlessly:
    measured whole-output rel err 1.05e-2 vs the 2e-2 budget.
  - AV: 3-product hi/lo split of both exp and v (0.75x, ~fp16 accuracy).

Layouts keep the contraction axis on SBUF partitions; no on-chip transposes:
  - qT, kT [D_part, N_free]  fp8, scaled by SW (projections computed
    transposed); exp scale absorbs 1/SW^2
  - scoresT computed transposed: scoresT[j_part, i_free]
  - v natural [N_part, D_free] hi/lo fp8, scaled by SW; the ones-column is SW
    so the row-sum column matches the SW-scaled numerator
  - exp stored hi/lo fp8 [N_part(j), N_free(i)]
"""

import math
import sys

import numpy as np

sys.path.insert(0, "/opt/trn_rl_repo")

import ml_dtypes  # noqa: E402

import concourse.bass as bass  # noqa: E402
import concourse.tile as tile  # noqa: E402
from concourse import bacc, mybir  # noqa: E402
from concourse import bass_utils  # noqa: E402

B, N, D = 8, 2048, 768
P = 128
DC = D // P  # 6 chunks of the embedding/contraction dim
PC = DC // 2  # 3 DoubleRow pair-chunks
NT = N // P  # 16 chunks of the sequence dim
NP2 = NT // 2  # 8 DoubleRow pair-chunks of the sequence dim
FD = 512  # matmul free-dim tile (one fp32 PSUM bank; ISA caps PSUM writes)
CDT = mybir.dt.float16
CDT_NP = np.float16
F8 = mybir.dt.float8e4
F8_NP = ml_dtypes.float8_e4m3
DR = mybir.MatmulPerfMode.DoubleRow
F32 = mybir.dt.float32
SW = 32.0  # host-side weight pre-scale (keeps W hi/lo splits in normal range)
INV_SQRT_D = 1.0 / math.sqrt(D)

# Filled by kernel() so a test harness can report the profiled HW time.
LAST_RESULT = None

# PSUM pool granularity: (tile_cols, bufs).
PSUM_GRAN = (1024, 4)


def _emit(tc, out, xh, xl, wqh, wql, wkh, wkl, wvh, wvl, bqk, bv, repeat=1):
    nc = tc.nc
    Ident = mybir.ActivationFunctionType.Identity
    Copy = mybir.ActivationFunctionType.Copy
    Exp = mybir.ActivationFunctionType.Exp
    Mult = mybir.AluOpType.mult
    Sub = mybir.AluOpType.subtract
    Add = mybir.AluOpType.add

    with (
        tc.tile_pool(name="const", bufs=1) as const,
        tc.tile_pool(name="data", bufs=1) as data,
        tc.tile_pool(name="expp", bufs=1) as expp,
        tc.tile_pool(name="tmpp", bufs=3) as tmpp,
        tc.tile_pool(name="psum", bufs=PSUM_GRAN[1], space="PSUM") as psum,
        tc.tile_pool(name="outp", bufs=3) as outp,
        tc.tile_pool(name="small", bufs=4) as small,
    ):
        # Persistent activations (all fp8; q/k/v carry the SW scale)
        qT = data.tile([P, DC, N], F8)  # qT[p, o, n] = SW*q[n, o*128+p]
        kT = data.tile([P, DC, N], F8)
        vh = data.tile([P, NT, D + 16], F8)  # v hi; col D = SW
        vl = data.tile([P, NT, D + 16], F8)  # v lo; col D = 0
        eh = expp.tile([P, NT, N], F8)  # exp hi: eh[p,t,i]~exp(s[i,t*128+p])
        el = expp.tile([P, NT, N], F8)  # exp lo (residual)
        xhs = data.tile([P, DC, N], F8)  # x hi, transposed: [d-part, n]
        xls = data.tile([P, DC, N], F8)  # x lo
        # wq is host-packed ec-major: wq*[p, ec, dc, c] = Wq'[dc*128+p,
        # ec*128+c] so the ec=0 stationary slices land in one small first DMA.
        wqhs = data.tile([P, DC, DC, P], F8)
        wqls = data.tile([P, DC, DC, P], F8)
        wkhs = data.tile([P, DC, D], F8)  # wk*[p, o, e] = Wk'[o*128+p, e]
        wkls = data.tile([P, DC, D], F8)
        wvhs = data.tile([P, DC, D], F8)
        wvls = data.tile([P, DC, D], F8)
        bqks = const.tile([P, 2 * DC], F32)  # host-packed: [p, o] = SW*bq,
        bqs = bqks[:, 0:DC]  # [p, DC+o] = SW*bk
        bks = bqks[:, DC : 2 * DC]
        bvb = const.tile([P, D], F32)  # bvb[p, e] = bv[e] (partition-bcast)

        def body():
            # Input loads in dependency order: the first projection needs wq +
            # x first; bvb is only read by the final epilogue.
            nc.sync.dma_start(wqhs[:, 0], wqh[0])
            nc.scalar.dma_start(wqls[:, 0], wql[0])
            nc.scalar.dma_start(bqks[:], bqk[:])
            H = N // 2
            for dc in range(DC):
                nc.sync.dma_start(
                    xhs[:, dc, 0:H],
                    xh[dc * P : (dc + 1) * P, 0:H].rearrange("(o p) n -> p o n", p=P),
                )
                nc.scalar.dma_start(
                    xls[:, dc, 0:H],
                    xl[dc * P : (dc + 1) * P, 0:H].rearrange("(o p) n -> p o n", p=P),
                )
                if dc < DC - 1:
                    nc.sync.dma_start(wqhs[:, dc + 1], wqh[dc + 1])
                    nc.scalar.dma_start(wqls[:, dc + 1], wql[dc + 1])
            for dc in range(DC):
                nc.sync.dma_start(
                    xhs[:, dc, H:N],
                    xh[dc * P : (dc + 1) * P, H:N].rearrange("(o p) n -> p o n", p=P),
                )
                nc.scalar.dma_start(
                    xls[:, dc, H:N],
                    xl[dc * P : (dc + 1) * P, H:N].rearrange("(o p) n -> p o n", p=P),
                )
            nc.sync.dma_start(wkhs[:], wkh.rearrange("(o p) e -> p o e", p=P))
            nc.scalar.dma_start(wkls[:], wkl.rearrange("(o p) e -> p o e", p=P))
            nc.sync.dma_start(wvhs[:], wvh.rearrange("(o p) e -> p o e", p=P))
            nc.scalar.dma_start(wvls[:], wvl.rearrange("(o p) e -> p o e", p=P))
            nc.scalar.dma_start(
                bvb[:],
                bass.AP(tensor=bv.tensor, offset=bv.offset, ap=[[0, P], *bv.ap]),
            )

            def mm3(ps, hi_pair_lhs, lo_pair_lhs, hi_pair_rhs, lo_pair_rhs,
                    first, last):
                """One 3-product hi/lo DoubleRow pair-chunk contraction:
                hi.hi + lo.hi + hi.lo accumulated into ps."""
                nc.tensor.matmul(ps, lhsT=hi_pair_lhs, rhs=hi_pair_rhs,
                                 start=first, stop=False, perf_mode=DR)
                nc.tensor.matmul(ps, lhsT=lo_pair_lhs, rhs=hi_pair_rhs,
                                 start=False, stop=False, perf_mode=DR)
                nc.tensor.matmul(ps, lhsT=hi_pair_lhs, rhs=lo_pair_rhs,
                                 start=False, stop=last, perf_mode=DR)

            # q, k projections in transposed layout:
            # qT[e, n] = SW*(sum_d Wq[d, e] * xT[d, n] + bq[e]) (e on parts)
            # q runs n-block-outermost so its first matmuls depend only on
            # wq[ec0] + the first half of x.
            G = PSUM_GRAN[0]
            for nb in range((N + G - 1) // G):
                for ec in range(DC):
                    ps = psum.tile([P, G], F32, tag="ps", name="ps")
                    for pc in range(PC):
                        d2 = slice(2 * pc, 2 * pc + 2)
                        for h in range(G // FD):
                            col = nb * G + h * FD
                            mm3(
                                ps[:, h * FD : (h + 1) * FD],
                                wqhs[:, ec, d2, :],
                                wqls[:, ec, d2, :],
                                xhs[:, d2, col : col + FD],
                                xls[:, d2, col : col + FD],
                                first=(pc == 0),
                                last=(pc == PC - 1),
                            )
                    nc.scalar.activation(
                        qT[:, ec, nb * G : (nb + 1) * G],
                        ps[:],
                        Ident,
                        bias=bqs[:, ec : ec + 1],
                    )
            for ec in range(DC):
                pss = [
                    psum.tile([P, G], F32, tag="ps", name="ps")
                    for _ in range(N // G)
                ]
                for pc in range(PC):
                    d2 = slice(2 * pc, 2 * pc + 2)
                    e2 = slice(ec * P, (ec + 1) * P)
                    for nj in range(N // FD):
                        ps = pss[nj // (G // FD)]
                        col = (nj % (G // FD)) * FD
                        mm3(
                            ps[:, col : col + FD],
                            wkhs[:, d2, e2],
                            wkls[:, d2, e2],
                            xhs[:, d2, nj * FD : (nj + 1) * FD],
                            xls[:, d2, nj * FD : (nj + 1) * FD],
                            first=(pc == 0),
                            last=(pc == PC - 1),
                        )
                for g, ps in enumerate(pss):
                    nc.scalar.activation(
                        kT[:, ec, g * G : (g + 1) * G],
                        ps[:],
                        Ident,
                        bias=bks[:, ec : ec + 1],
                    )

            # Merged scores + v-projection loop (1:1 over the 16 seq tiles).
            # The v matmuls keep PE fed while ACT/DVE/GPSIMD chew through the
            # exp hi/lo splitting, which would otherwise starve PE (the fp8
            # scores matmul is the cheapest PE phase but carries the heaviest
            # element-wise load).
            #
            # scoresT[j, i] = sum_d kT[d, j] * qT[d, i] (pure fp8 DoubleRow);
            # exp with 1/(SW^2 sqrt(D)) folded into the activation scale.
            # exp lands in a rotating fp16 buffer, split into fp8 hi (DVE) +
            # lo (GPSIMD) for the AV 3-product.
            #
            # v[n, e] = SW * sum_d x[n, d] * Wv[d, e] (bias deferred to the
            # epilogue); hi from ACT, lo residual from DVE. Column D gets SW
            # (hi) / 0 (lo) so the AV matmul also produces SW-scaled softmax
            # row sums.
            for jt in range(NT):
                pss = [
                    psum.tile([P, G], F32, tag="ps", name="ps")
                    for _ in range(N // G)
                ]
                for pc in range(PC):
                    d2 = slice(2 * pc, 2 * pc + 2)
                    lhsT = kT[:, d2, jt * P : (jt + 1) * P]
                    for ni in range(N // FD):
                        ps = pss[ni // (G // FD)]
                        col = (ni % (G // FD)) * FD
                        nc.tensor.matmul(
                            ps[:, col : col + FD],
                            lhsT=lhsT,
                            rhs=qT[:, d2, ni * FD : (ni + 1) * FD],
                            start=(pc == 0),
                            stop=(pc == PC - 1),
                            perf_mode=DR,
                        )
                nt = jt
                vps = psum.tile([P, PSUM_GRAN[0]], F32, tag="ps", name="ps")
                n2 = slice(nt * P, (nt + 1) * P)
                for pc in range(PC):
                    d2 = slice(2 * pc, 2 * pc + 2)
                    mm3(
                        vps[:, 0:FD],
                        xhs[:, d2, n2], xls[:, d2, n2],
                        wvhs[:, d2, 0:FD], wvls[:, d2, 0:FD],
                        first=(pc == 0), last=(pc == PC - 1),
                    )
                    mm3(
                        vps[:, FD:D],
                        xhs[:, d2, n2], xls[:, d2, n2],
                        wvhs[:, d2, FD:D], wvls[:, d2, FD:D],
                        first=(pc == 0), last=(pc == PC - 1),
                    )
                et = tmpp.tile([P, N], CDT, tag="et", name="et")
                for g, ps in enumerate(pss):
                    nc.scalar.activation(
                        et[:, g * G : (g + 1) * G],
                        ps[:],
                        Exp,
                        scale=INV_SQRT_D / (SW * SW),
                    )
                nc.vector.tensor_copy(eh[:, jt], et[:])
                nc.gpsimd.tensor_sub(el[:, jt], et[:], eh[:, jt])
                nc.scalar.activation(vh[:, nt, 0:D], vps[:, 0:D], Copy)
                nc.vector.scalar_tensor_tensor(
                    vl[:, nt, 0:D], vps[:, 0:D], 1.0, vh[:, nt, 0:D],
                    op0=Mult, op1=Sub,
                )
                nc.vector.memset(vh[:, nt, D : D + 1], SW)
                nc.vector.memset(vl[:, nt, D : D + 1], 0.0)

            # out[i, e] = sum_j exp[j, i] * v[j, e] via the hi/lo 3-product;
            # col D accumulates SW * row sums (SW cancels in the division).
            for it in range(NT):
                ps = psum.tile([P, PSUM_GRAN[0]], F32, tag="ps", name="ps")
                i2 = slice(it * P, (it + 1) * P)
                for jp in range(NP2):
                    j2 = slice(2 * jp, 2 * jp + 2)
                    mm3(
                        ps[:, 0:FD],
                        eh[:, j2, i2], el[:, j2, i2],
                        vh[:, j2, 0:FD], vl[:, j2, 0:FD],
                        first=(jp == 0), last=(jp == NP2 - 1),
                    )
                    mm3(
                        ps[:, FD : D + 1],
                        eh[:, j2, i2], el[:, j2, i2],
                        vh[:, j2, FD : D + 1], vl[:, j2, FD : D + 1],
                        first=(jp == 0), last=(jp == NP2 - 1),
                    )
                recip = small.tile([P, 1], F32, tag="recip", name="recip")
                nc.vector.reciprocal(recip[:], ps[:, D : D + 1])
                # fp16 store (half the output DMA); host upcasts to f32.
                of = outp.tile([P, D], CDT, tag="of", name="of")
                nc.vector.scalar_tensor_tensor(
                    of[:],
                    ps[:, 0:D],
                    recip[:],
                    bvb[:],
                    op0=Mult,
                    op1=Add,
                )
                nc.sync.dma_start(out[it * P : (it + 1) * P, :], of[:])

        if repeat == 1:
            body()
        else:
            hints = (
                mybir.EngineType.PE,
                mybir.EngineType.Activation,
                mybir.EngineType.DVE,
                mybir.EngineType.Pool,
                mybir.EngineType.SP,
            )
            with tc.For_i(0, repeat, 1, hint_engines=hints):
                body()


def _build(repeat=1):
    nc = bacc.Bacc(
        "TRN2",
        target_bir_lowering=False,
        debug=False,
        enable_asserts=False,
        num_devices=B,
    )
    xh = nc.dram_tensor("xh", [D, N], F8, kind="ExternalInput").ap()
    xl = nc.dram_tensor("xl", [D, N], F8, kind="ExternalInput").ap()
    wqh = nc.dram_tensor("wqh", [DC, P, DC, P], F8, kind="ExternalInput").ap()
    wql = nc.dram_tensor("wql", [DC, P, DC, P], F8, kind="ExternalInput").ap()
    wkh = nc.dram_tensor("wkh", [D, D], F8, kind="ExternalInput").ap()
    wkl = nc.dram_tensor("wkl", [D, D], F8, kind="ExternalInput").ap()
    wvh = nc.dram_tensor("wvh", [D, D], F8, kind="ExternalInput").ap()
    wvl = nc.dram_tensor("wvl", [D, D], F8, kind="ExternalInput").ap()
    bqk = nc.dram_tensor("bqk", [P, 2 * DC], F32, kind="ExternalInput").ap()
    bv = nc.dram_tensor("bv", [D], F32, kind="ExternalInput").ap()
    out = nc.dram_tensor("out", [N, D], CDT, kind="ExternalOutput").ap()
    with tile.TileContext(nc) as tc:
        _emit(tc, out, xh, xl, wqh, wql, wkh, wkl, wvh, wvl, bqk, bv,
              repeat=repeat)
    nc.compile()
    return nc


def _split8(a):
    """Split fp32/64 array into e4m3 hi + lo with hi = rne(a)."""
    hi = a.astype(F8_NP)
    lo = (a - hi.astype(np.float32)).astype(F8_NP)
    return hi, lo


def make_in_maps(inputs):
    x = np.asarray(inputs["x"], np.float32)
    wq = np.asarray(inputs["Wq"], np.float32) * SW
    wk = np.asarray(inputs["Wk"], np.float32) * SW
    wv = np.asarray(inputs["Wv"], np.float32) * SW
    # ec-major packing: wq2[ec, p, dc, c] = Wq'[dc*128+p, ec*128+c]
    def qpack(w):
        return np.ascontiguousarray(
            w.reshape(DC, P, DC, P).transpose(2, 1, 0, 3))
    wqh, wql = _split8(qpack(wq))
    wkh, wkl = _split8(wk)
    wvh, wvl = _split8(wv)
    bq = np.asarray(inputs["bq"], np.float32) * SW
    bk = np.asarray(inputs["bk"], np.float32) * SW
    bv = np.ascontiguousarray(np.asarray(inputs["bv"], np.float32))
    # bqk[p, o] = SW*bq[o*128+p]; bqk[p, DC+o] = SW*bk[o*128+p]
    bqk = np.ascontiguousarray(
        np.concatenate([bq.reshape(DC, P).T, bk.reshape(DC, P).T], axis=1)
    )
    maps = []
    for b in range(B):
        xT = np.ascontiguousarray(x[b].T)
        xh, xl = _split8(xT)
        maps.append(
            {
                "xh": xh,
                "xl": xl,
                "wqh": wqh,
                "wql": wql,
                "wkh": wkh,
                "wkl": wkl,
                "wvh": wvh,
                "wvl": wvl,
                "bqk": bqk,
                "bv": bv,
            }
        )
    return maps


_NC_CACHE = {}


def kernel(**inputs):
    global LAST_RESULT
    in_maps = make_in_maps(inputs)

    if 1 not in _NC_CACHE:
        _NC_CACHE[1] = _build()
    nc = _NC_CACHE[1]
    res = None
    for attempt in range(3):
        try:
            res = bass_utils.run_bass_kernel_spmd(nc, in_maps, core_ids=list(range(B)))
            break
        except Exception:
            if attempt == 2:
                raise
    LAST_RESULT = res
    return np.stack([res.results[c]["out"] for c in range(B)], axis=0).astype(
        np.float32
    )


if __name__ == "__main__":
    rng = np.random.default_rng(0)
    demo = {
        "x": rng.standard_normal((B, N, D), dtype=np.float32),
        "Wq": rng.uniform(-0.036, 0.036, (D, D)).astype(np.float32),
        "bq": rng.uniform(-0.036, 0.036, D).astype(np.float32),
        "Wk": rng.uniform(-0.036, 0.036, (D, D)).astype(np.float32),
        "bk": rng.uniform(-0.036, 0.036, D).astype(np.float32),
        "Wv": rng.uniform(-0.036, 0.036, (D, D)).astype(np.float32),
        "bv": rng.uniform(-0.036, 0.036, D).astype(np.float32),
    }
    out = kernel(**demo)
    print("out", out.shape, out.dtype, float(np.abs(out).max()))


# revision 13
# speedup vs baseline: 1.3471x; 1.2564x over previous
"""Single-head attention block (B=8, N=2048, D=768) on 8 Trainium2 NeuronCores.

Strategy: pure data-parallel over the batch dimension — one batch element per
NeuronCore. Each core computes, for its x_b [N, D]:

  q = x@Wq + bq, k = x@Wk + bk, v = x@Wv        (fp16 matmuls, fp32 PSUM accum)
  expT[j, i] = exp((q_i . k_j) / sqrt(D))        (no max-subtraction: scaled
                                                  scores are bounded ~|2.2|)
  outU[i, :] = sum_j expT[j, i] * v_aug[j, :]    (v_aug has a ones column ->
                                                  col D holds the softmax row
                                                  sums)
  out[i, e] = outU[i, e] / outU[i, D] + bv[e]    (v-bias folded to the end:
                                                  softmax rows sum to 1)

Layouts are chosen so the softmax contraction axis (j) always sits on SBUF
partitions and no on-chip transposes are ever needed:
  - qT, kT [D_part, N_free]   (projections computed transposed)
  - scores computed transposed: scoresT[j_part, i_free]
  - v natural [N_part, D_free] which is exactly the AV matmul's moving operand
"""

import math
import sys

import numpy as np

sys.path.insert(0, "/opt/trn_rl_repo")

import ml_dtypes  # noqa: E402

import concourse.bass as bass  # noqa: E402
import concourse.tile as tile  # noqa: E402
from concourse import bacc, mybir  # noqa: E402
from concourse import bass_utils  # noqa: E402

B, N, D = 8, 2048, 768
P = 128
DC = D // P  # 6 chunks of the embedding/contraction dim
NT = N // P  # 16 chunks of the sequence dim
FD = 512  # matmul free-dim tile (one fp32 PSUM bank; ISA caps PSUM writes)
# Compute dtype for matmul operands: fp16 runs at the same PE rate as bf16
# but carries 10 mantissa bits instead of 7 (L2 err 2.8e-4 vs 2.2e-3).
CDT = mybir.dt.float16
CDT_NP = np.float16
F8 = mybir.dt.float8e4
DR = mybir.MatmulPerfMode.DoubleRow
F32 = mybir.dt.float32
INV_SQRT_D = 1.0 / math.sqrt(D)

# Filled by kernel() so a test harness can report the profiled HW time.
LAST_RESULT = None

# PSUM pool granularity: (tile_cols, bufs). (2048, 2) = two 4-bank tiles;
# (1024, 4) = four 2-bank tiles (finer pipelining, more ACT instructions).
PSUM_GRAN = (1024, 4)


def _emit(tc, out, xT, wq, wk, wv, bqk, bv, repeat=1):
    nc = tc.nc
    Ident = mybir.ActivationFunctionType.Identity
    Copy = mybir.ActivationFunctionType.Copy
    Exp = mybir.ActivationFunctionType.Exp

    with (
        tc.tile_pool(name="const", bufs=1) as const,
        tc.tile_pool(name="data", bufs=1) as data,
        tc.tile_pool(name="expp", bufs=1) as expp,
        tc.tile_pool(name="psum", bufs=PSUM_GRAN[1], space="PSUM") as psum,
        tc.tile_pool(name="outp", bufs=3) as outp,
        tc.tile_pool(name="small", bufs=4) as small,
    ):
        # Persistent activations
        qT = data.tile([P, DC, N], F8)  # qT[p, o, n] = q[n, o*128+p]
        kT = data.tile([P, DC, N], F8)
        v = data.tile([P, NT, D + 16], CDT)  # v[p, t, e] = v[t*128+p, e]; col D = 1.0
        expT = expp.tile([P, NT, N], CDT)  # expT[p, t, i] = exp(s[i, t*128+p]/sqrt(D))
        xTs = data.tile([P, DC, N], CDT)  # xTs[p, o, n] = x[n, o*128+p]
        # wq is host-packed ec-major: wqs[p, ec, dc, c] = Wq[dc*128+p, ec*128+c],
        # so the ec=0 stationary slices land in one small first DMA.
        wqs = data.tile([P, DC, DC, P], CDT)
        wks = data.tile([P, DC, D], CDT)  # wks[p, o, e] = Wk[o*128+p, e]
        wvs = data.tile([P, DC, D], CDT)
        bqks = const.tile([P, 2 * DC], F32)  # host-packed: bqks[p, o] = bq[o*128+p],
        bqs = bqks[:, 0:DC]  # bqks[p, DC+o] = bk[o*128+p]
        bks = bqks[:, DC : 2 * DC]
        bvb = const.tile([P, D], F32)  # bvb[p, e] = bv[e] (partition-broadcast)

        def body():
            # Input loads in dependency order: the first projection needs wq +
            # xT first; bvb is only read by the final epilogue. bqk is packed
            # host-side into per-partition layout so it is one tiny
            # 128-descriptor DMA instead of a 768x4B gather.
            nc.sync.dma_start(wqs[:, 0], wq[0])
            nc.scalar.dma_start(bqks[:], bqk[:])
            H = N // 2
            for dc in range(DC):
                nc.sync.dma_start(
                    xTs[:, dc, 0:H],
                    xT[dc * P : (dc + 1) * P, 0:H].rearrange(
                        "(o p) n -> p o n", p=P
                    ),
                )
                if dc < DC - 1:
                    nc.sync.dma_start(wqs[:, dc + 1], wq[dc + 1])
            for dc in range(DC):
                nc.sync.dma_start(
                    xTs[:, dc, H:N],
                    xT[dc * P : (dc + 1) * P, H:N].rearrange(
                        "(o p) n -> p o n", p=P
                    ),
                )
            nc.sync.dma_start(wks[:], wk.rearrange("(o p) e -> p o e", p=P))
            nc.sync.dma_start(wvs[:], wv.rearrange("(o p) e -> p o e", p=P))
            nc.scalar.dma_start(
                bvb[:],
                bass.AP(tensor=bv.tensor, offset=bv.offset, ap=[[0, P], *bv.ap]),
            )

            # q, k projections in transposed layout:
            # qT[e, n] = sum_d Wq[d, e] * xT[d, n], then + bq[e] (e on partitions)
            # q runs n-block-outermost so its first matmuls depend only on
            # wq[ec0] + the first half of xT (early PE start while the rest
            # of the inputs stream in).
            G = PSUM_GRAN[0]
            for nb in range((N + G - 1) // G):
                for ec in range(DC):
                    ps = psum.tile([P, G], F32, tag="ps", name="ps")
                    for dc in range(DC):
                        lhsT = wqs[:, ec, dc, :]
                        for h in range(G // FD):
                            col = h * FD
                            nc.tensor.matmul(
                                ps[:, col : col + FD],
                                lhsT=lhsT,
                                rhs=xTs[:, dc, nb * G + col : nb * G + col + FD],
                                start=(dc == 0),
                                stop=(dc == DC - 1),
                            )
                    nc.scalar.activation(
                        qT[:, ec, nb * G : (nb + 1) * G],
                        ps[:],
                        Ident,
                        bias=bqs[:, ec : ec + 1],
                    )
            for ec in range(DC):
                pss = [
                    psum.tile([P, G], F32, tag="ps", name="ps")
                    for _ in range(N // G)
                ]
                for dc in range(DC):
                    lhsT = wks[:, dc, ec * P : (ec + 1) * P]
                    for nj in range(N // FD):
                        ps = pss[nj // (G // FD)]
                        col = (nj % (G // FD)) * FD
                        nc.tensor.matmul(
                            ps[:, col : col + FD],
                            lhsT=lhsT,
                            rhs=xTs[:, dc, nj * FD : (nj + 1) * FD],
                            start=(dc == 0),
                            stop=(dc == DC - 1),
                        )
                for g, ps in enumerate(pss):
                    nc.scalar.activation(
                        kT[:, ec, g * G : (g + 1) * G],
                        ps[:],
                        Ident,
                        bias=bks[:, ec : ec + 1],
                    )

            # v projection in natural layout: v[n, e] = sum_d xT[d, n] * Wv[d, e]
            # (bias deferred to the epilogue). Column D gets 1.0 so the AV
            # matmul also produces softmax row sums.
            for nt in range(NT):
                ps = psum.tile([P, PSUM_GRAN[0]], F32, tag="ps", name="ps")
                for dc in range(DC):
                    lhsT = xTs[:, dc, nt * P : (nt + 1) * P]
                    nc.tensor.matmul(
                        ps[:, 0:FD],
                        lhsT=lhsT,
                        rhs=wvs[:, dc, 0:FD],
                        start=(dc == 0),
                        stop=(dc == DC - 1),
                    )
                    nc.tensor.matmul(
                        ps[:, FD:D],
                        lhsT=lhsT,
                        rhs=wvs[:, dc, FD:D],
                        start=(dc == 0),
                        stop=(dc == DC - 1),
                    )
                nc.scalar.activation(v[:, nt, 0:D], ps[:, 0:D], Copy)
                nc.vector.memset(v[:, nt, D : D + 1], 1.0)

            # scoresT[j, i] = sum_d kT[d, j] * qT[d, i]; exp with the 1/sqrt(D)
            # scale folded into the activation.
            for jt in range(NT):
                pss = [
                    psum.tile([P, G], F32, tag="ps", name="ps")
                    for _ in range(N // G)
                ]
                for pc in range(DC // 2):
                    lhsT = kT[:, 2 * pc : 2 * pc + 2, jt * P : (jt + 1) * P]
                    for ni in range(N // FD):
                        ps = pss[ni // (G // FD)]
                        col = (ni % (G // FD)) * FD
                        nc.tensor.matmul(
                            ps[:, col : col + FD],
                            lhsT=lhsT,
                            rhs=qT[:, 2 * pc : 2 * pc + 2, ni * FD : (ni + 1) * FD],
                            start=(pc == 0),
                            stop=(pc == DC // 2 - 1),
                            perf_mode=DR,
                        )
                for g, ps in enumerate(pss):
                    nc.scalar.activation(
                        expT[:, jt, g * G : (g + 1) * G],
                        ps[:],
                        Exp,
                        scale=INV_SQRT_D,
                    )

            # out[i, e] = sum_j expT[j, i] * v[j, e]; col D accumulates row sums.
            for it in range(NT):
                ps = psum.tile([P, PSUM_GRAN[0]], F32, tag="ps", name="ps")
                for jt in range(NT):
                    lhsT = expT[:, jt, it * P : (it + 1) * P]
                    nc.tensor.matmul(
                        ps[:, 0:FD],
                        lhsT=lhsT,
                        rhs=v[:, jt, 0:FD],
                        start=(jt == 0),
                        stop=(jt == NT - 1),
                    )
                    nc.tensor.matmul(
                        ps[:, FD : D + 1],
                        lhsT=lhsT,
                        rhs=v[:, jt, FD : D + 1],
                        start=(jt == 0),
                        stop=(jt == NT - 1),
                    )
                recip = small.tile([P, 1], F32, tag="recip", name="recip")
                nc.vector.reciprocal(recip[:], ps[:, D : D + 1])
                of = outp.tile([P, D], CDT, tag="of", name="of")
                nc.vector.scalar_tensor_tensor(
                    of[:],
                    ps[:, 0:D],
                    recip[:],
                    bvb[:],
                    op0=mybir.AluOpType.mult,
                    op1=mybir.AluOpType.add,
                )
                nc.sync.dma_start(out[it * P : (it + 1) * P, :], of[:])

        if repeat == 1:
            body()
        else:
            hints = (
                mybir.EngineType.PE,
                mybir.EngineType.Activation,
                mybir.EngineType.DVE,
                mybir.EngineType.SP,
            )
            with tc.For_i(0, repeat, 1, hint_engines=hints):
                body()


def _build(repeat=1):
    nc = bacc.Bacc(
        "TRN2",
        target_bir_lowering=False,
        debug=False,
        enable_asserts=False,
        num_devices=B,
    )
    xT = nc.dram_tensor("xT", [D, N], CDT, kind="ExternalInput").ap()
    wq = nc.dram_tensor("wq", [DC, P, DC, P], CDT, kind="ExternalInput").ap()
    wk = nc.dram_tensor("wk", [D, D], CDT, kind="ExternalInput").ap()
    wv = nc.dram_tensor("wv", [D, D], CDT, kind="ExternalInput").ap()
    bqk = nc.dram_tensor("bqk", [P, 2 * DC], F32, kind="ExternalInput").ap()
    bv = nc.dram_tensor("bv", [D], F32, kind="ExternalInput").ap()
    out = nc.dram_tensor("out", [N, D], CDT, kind="ExternalOutput").ap()
    with tile.TileContext(nc) as tc:
        _emit(tc, out, xT, wq, wk, wv, bqk, bv, repeat=repeat)
    nc.compile()
    return nc


def make_in_maps(inputs):
    x = np.asarray(inputs["x"], np.float32)
    bf = CDT_NP
    wq = np.asarray(inputs["Wq"], np.float32).astype(bf)
    # ec-major packing: wq2[ec, p, dc, c] = Wq[dc*128+p, ec*128+c]
    wq2 = np.ascontiguousarray(wq.reshape(DC, P, DC, P).transpose(2, 1, 0, 3))
    wk = np.asarray(inputs["Wk"], np.float32).astype(bf)
    wv = np.asarray(inputs["Wv"], np.float32).astype(bf)
    bq = np.asarray(inputs["bq"], np.float32)
    bk = np.asarray(inputs["bk"], np.float32)
    bv = np.ascontiguousarray(np.asarray(inputs["bv"], np.float32))
    # bqk[p, o] = bq[o*128+p]; bqk[p, DC+o] = bk[o*128+p]
    bqk = np.ascontiguousarray(
        np.concatenate([bq.reshape(DC, P).T, bk.reshape(DC, P).T], axis=1)
    )
    return [
        {
            "xT": np.ascontiguousarray(x[b].T).astype(bf),
            "wq": wq2,
            "wk": wk,
            "wv": wv,
            "bqk": bqk,
            "bv": bv,
        }
        for b in range(B)
    ]


_NC_CACHE = {}


def kernel(**inputs):
    global LAST_RESULT
    in_maps = make_in_maps(inputs)

    if 1 not in _NC_CACHE:
        _NC_CACHE[1] = _build()
    nc = _NC_CACHE[1]
    res = None
    for attempt in range(3):
        try:
            res = bass_utils.run_bass_kernel_spmd(nc, in_maps, core_ids=list(range(B)))
            break
        except Exception:
            if attempt == 2:
                raise
    LAST_RESULT = res
    return np.stack([res.results[c]["out"] for c in range(B)], axis=0).astype(np.float32)


if __name__ == "__main__":
    rng = np.random.default_rng(0)
    demo = {
        "x": rng.standard_normal((B, N, D), dtype=np.float32),
        "Wq": rng.uniform(-0.036, 0.036, (D, D)).astype(np.float32),
        "bq": rng.uniform(-0.036, 0.036, D).astype(np.float32),
        "Wk": rng.uniform(-0.036, 0.036, (D, D)).astype(np.float32),
        "bk": rng.uniform(-0.036, 0.036, D).astype(np.float32),
        "Wv": rng.uniform(-0.036, 0.036, (D, D)).astype(np.float32),
        "bv": rng.uniform(-0.036, 0.036, D).astype(np.float32),
    }
    out = kernel(**demo)
    print("out", out.shape, out.dtype, float(np.abs(out).max()))



# revision 15
# speedup vs baseline: 1.4235x; 1.0567x over previous
"""Single-head attention block (B=8, N=2048, D=768) on 8 Trainium2 NeuronCores.

Strategy: pure data-parallel over the batch dimension — one batch element per
NeuronCore. Each core computes, for its x_b [N, D]:

  q = x@Wq + bq, k = x@Wk + bk, v = x@Wv        (fp16 matmuls, fp32 PSUM accum)
  expT[j, i] = exp((q_i . k_j) / sqrt(D))        (no max-subtraction: scaled
                                                  scores are bounded ~|2.2|)
  outU[i, :] = sum_j expT[j, i] * v_aug[j, :]    (v_aug has a ones column ->
                                                  col D holds the softmax row
                                                  sums)
  out[i, e] = outU[i, e] / outU[i, D] + bv[e]    (v-bias folded to the end:
                                                  softmax rows sum to 1)

Layouts are chosen so the softmax contraction axis (j) always sits on SBUF
partitions and no on-chip transposes are ever needed:
  - qT, kT [D_part, N_free]   (projections computed transposed)
  - scores computed transposed: scoresT[j_part, i_free]
  - v natural [N_part, D_free] which is exactly the AV matmul's moving operand
"""

import math
import sys

import numpy as np

sys.path.insert(0, "/opt/trn_rl_repo")

import ml_dtypes  # noqa: E402

import concourse.bass as bass  # noqa: E402
import concourse.tile as tile  # noqa: E402
from concourse import bacc, mybir  # noqa: E402
from concourse import bass_utils  # noqa: E402

B, N, D = 8, 2048, 768
P = 128
DC = D // P  # 6 chunks of the embedding/contraction dim
NT = N // P  # 16 chunks of the sequence dim
FD = 512  # matmul free-dim tile (one fp32 PSUM bank; ISA caps PSUM writes)
# Compute dtype for matmul operands: fp16 runs at the same PE rate as bf16
# but carries 10 mantissa bits instead of 7 (L2 err 2.8e-4 vs 2.2e-3).
CDT = mybir.dt.float16
CDT_NP = np.float16
F8 = mybir.dt.float8e4
DR = mybir.MatmulPerfMode.DoubleRow
F32 = mybir.dt.float32
INV_SQRT_D = 1.0 / math.sqrt(D)

# Filled by kernel() so a test harness can report the profiled HW time.
LAST_RESULT = None

# PSUM pool granularity: (tile_cols, bufs). (2048, 2) = two 4-bank tiles;
# (1024, 4) = four 2-bank tiles (finer pipelining, more ACT instructions).
PSUM_GRAN = (1024, 4)


def _emit(tc, out, xT, wq, wk, wv, bqk, bv, repeat=1):
    nc = tc.nc
    Ident = mybir.ActivationFunctionType.Identity
    Copy = mybir.ActivationFunctionType.Copy
    Exp = mybir.ActivationFunctionType.Exp

    with (
        tc.tile_pool(name="const", bufs=1) as const,
        tc.tile_pool(name="data", bufs=1) as data,
        tc.tile_pool(name="expp", bufs=1) as expp,
        tc.tile_pool(name="psum", bufs=PSUM_GRAN[1], space="PSUM") as psum,
        tc.tile_pool(name="outp", bufs=3) as outp,
        tc.tile_pool(name="small", bufs=4) as small,
    ):
        # Persistent activations
        qT = data.tile([P, DC, N], F8)  # qT[p, o, n] = q[n, o*128+p]
        kT = data.tile([P, DC, N], F8)
        v = data.tile([P, NT, D + 16], CDT)  # v[p, t, e] = v[t*128+p, e]; col D = 1.0
        expT = expp.tile([P, NT, N], CDT)  # expT[p, t, i] = exp(s[i, t*128+p]/sqrt(D))
        xTs = data.tile([P, DC, N], CDT)  # xTs[p, o, n] = x[n, o*128+p]
        # wq is host-packed ec-major: wqs[p, ec, dc, c] = Wq[dc*128+p, ec*128+c],
        # so the ec=0 stationary slices land in one small first DMA.
        wqs = data.tile([P, DC, DC, P], CDT)
        wks = data.tile([P, DC, D], CDT)  # wks[p, o, e] = Wk[o*128+p, e]
        wvs = data.tile([P, DC, D], CDT)
        bqks = const.tile([P, 2 * DC], F32)  # host-packed: bqks[p, o] = bq[o*128+p],
        bqs = bqks[:, 0:DC]  # bqks[p, DC+o] = bk[o*128+p]
        bks = bqks[:, DC : 2 * DC]
        bvb = const.tile([P, D], F32)  # bvb[p, e] = bv[e] (partition-broadcast)

        def body():
            # Input loads in dependency order: the first projection needs wq +
            # xT first; bvb is only read by the final epilogue. bqk is packed
            # host-side into per-partition layout so it is one tiny
            # 128-descriptor DMA instead of a 768x4B gather.
            nc.sync.dma_start(wqs[:, 0], wq[0])
            nc.scalar.dma_start(bqks[:], bqk[:])
            H = N // 2
            for dc in range(DC):
                nc.sync.dma_start(
                    xTs[:, dc, 0:H],
                    xT[dc * P : (dc + 1) * P, 0:H].rearrange(
                        "(o p) n -> p o n", p=P
                    ),
                )
                if dc < DC - 1:
                    nc.sync.dma_start(wqs[:, dc + 1], wq[dc + 1])
            for dc in range(DC):
                nc.sync.dma_start(
                    xTs[:, dc, H:N],
                    xT[dc * P : (dc + 1) * P, H:N].rearrange(
                        "(o p) n -> p o n", p=P
                    ),
                )
            nc.sync.dma_start(wks[:], wk.rearrange("(o p) e -> p o e", p=P))
            nc.sync.dma_start(wvs[:], wv.rearrange("(o p) e -> p o e", p=P))
            nc.scalar.dma_start(
                bvb[:],
                bass.AP(tensor=bv.tensor, offset=bv.offset, ap=[[0, P], *bv.ap]),
            )

            # q, k projections in transposed layout:
            # qT[e, n] = sum_d Wq[d, e] * xT[d, n], then + bq[e] (e on partitions)
            # q runs n-block-outermost so its first matmuls depend only on
            # wq[ec0] + the first half of xT (early PE start while the rest
            # of the inputs stream in).
            G = PSUM_GRAN[0]
            for nb in range((N + G - 1) // G):
                for ec in range(DC):
                    ps = psum.tile([P, G], F32, tag="ps", name="ps")
                    for dc in range(DC):
                        lhsT = wqs[:, ec, dc, :]
                        for h in range(G // FD):
                            col = h * FD
                            nc.tensor.matmul(
                                ps[:, col : col + FD],
                                lhsT=lhsT,
                                rhs=xTs[:, dc, nb * G + col : nb * G + col + FD],
                                start=(dc == 0),
                                stop=(dc == DC - 1),
                            )
                    nc.scalar.activation(
                        qT[:, ec, nb * G : (nb + 1) * G],
                        ps[:],
                        Ident,
                        bias=bqs[:, ec : ec + 1],
                    )
            for ec in range(DC):
                pss = [
                    psum.tile([P, G], F32, tag="ps", name="ps")
                    for _ in range(N // G)
                ]
                for dc in range(DC):
                    lhsT = wks[:, dc, ec * P : (ec + 1) * P]
                    for nj in range(N // FD):
                        ps = pss[nj // (G // FD)]
                        col = (nj % (G // FD)) * FD
                        nc.tensor.matmul(
                            ps[:, col : col + FD],
                            lhsT=lhsT,
                            rhs=xTs[:, dc, nj * FD : (nj + 1) * FD],
                            start=(dc == 0),
                            stop=(dc == DC - 1),
                        )
                for g, ps in enumerate(pss):
                    nc.scalar.activation(
                        kT[:, ec, g * G : (g + 1) * G],
                        ps[:],
                        Ident,
                        bias=bks[:, ec : ec + 1],
                    )

            # v projection in natural layout: v[n, e] = sum_d xT[d, n] * Wv[d, e]
            # (bias deferred to the epilogue). Column D gets 1.0 so the AV
            # matmul also produces softmax row sums.
            for nt in range(NT):
                ps = psum.tile([P, PSUM_GRAN[0]], F32, tag="ps", name="ps")
                for dc in range(DC):
                    lhsT = xTs[:, dc, nt * P : (nt + 1) * P]
                    nc.tensor.matmul(
                        ps[:, 0:FD],
                        lhsT=lhsT,
                        rhs=wvs[:, dc, 0:FD],
                        start=(dc == 0),
                        stop=(dc == DC - 1),
                    )
                    nc.tensor.matmul(
                        ps[:, FD:D],
                        lhsT=lhsT,
                        rhs=wvs[:, dc, FD:D],
                        start=(dc == 0),
                        stop=(dc == DC - 1),
                    )
                nc.scalar.activation(v[:, nt, 0:D], ps[:, 0:D], Copy)
                nc.vector.memset(v[:, nt, D : D + 1], 1.0)

            # scoresT[j, i] = sum_d kT[d, j] * qT[d, i]; exp with the 1/sqrt(D)
            # scale folded into the activation.
            for jt in range(NT):
                pss = [
                    psum.tile([P, G], F32, tag="ps", name="ps")
                    for _ in range(N // G)
                ]
                for pc in range(DC // 2):
                    lhsT = kT[:, 2 * pc : 2 * pc + 2, jt * P : (jt + 1) * P]
                    for ni in range(N // FD):
                        ps = pss[ni // (G // FD)]
                        col = (ni % (G // FD)) * FD
                        nc.tensor.matmul(
                            ps[:, col : col + FD],
                            lhsT=lhsT,
                            rhs=qT[:, 2 * pc : 2 * pc + 2, ni * FD : (ni + 1) * FD],
                            start=(pc == 0),
                            stop=(pc == DC // 2 - 1),
                            perf_mode=DR,
                        )
                for g, ps in enumerate(pss):
                    nc.scalar.activation(
                        expT[:, jt, g * G : (g + 1) * G],
                        ps[:],
                        Exp,
                        scale=INV_SQRT_D,
                    )

            # out[i, e] = sum_j expT[j, i] * v[j, e]; col D accumulates row sums.
            for it in range(NT):
                ps = psum.tile([P, PSUM_GRAN[0]], F32, tag="ps", name="ps")
                for jt in range(NT):
                    lhsT = expT[:, jt, it * P : (it + 1) * P]
                    nc.tensor.matmul(
                        ps[:, 0:FD],
                        lhsT=lhsT,
                        rhs=v[:, jt, 0:FD],
                        start=(jt == 0),
                        stop=(jt == NT - 1),
                    )
                    nc.tensor.matmul(
                        ps[:, FD : D + 1],
                        lhsT=lhsT,
                        rhs=v[:, jt, FD : D + 1],
                        start=(jt == 0),
                        stop=(jt == NT - 1),
                    )
                recip = small.tile([P, 1], F32, tag="recip", name="recip")
                nc.vector.reciprocal(recip[:], ps[:, D : D + 1])
                of = outp.tile([P, D], CDT, tag="of", name="of")
                nc.vector.scalar_tensor_tensor(
                    of[:],
                    ps[:, 0:D],
                    recip[:],
                    bvb[:],
                    op0=mybir.AluOpType.mult,
                    op1=mybir.AluOpType.add,
                )
                nc.sync.dma_start(out[it * P : (it + 1) * P, :], of[:])

        if repeat == 1:
            body()
        else:
            hints = (
                mybir.EngineType.PE,
                mybir.EngineType.Activation,
                mybir.EngineType.DVE,
                mybir.EngineType.SP,
            )
            with tc.For_i(0, repeat, 1, hint_engines=hints):
                body()


def _build(repeat=1):
    nc = bacc.Bacc(
        "TRN2",
        target_bir_lowering=False,
        debug=False,
        enable_asserts=False,
        num_devices=B,
    )
    xT = nc.dram_tensor("xT", [D, N], CDT, kind="ExternalInput").ap()
    wq = nc.dram_tensor("wq", [DC, P, DC, P], CDT, kind="ExternalInput").ap()
    wk = nc.dram_tensor("wk", [D, D], CDT, kind="ExternalInput").ap()
    wv = nc.dram_tensor("wv", [D, D], CDT, kind="ExternalInput").ap()
    bqk = nc.dram_tensor("bqk", [P, 2 * DC], F32, kind="ExternalInput").ap()
    bv = nc.dram_tensor("bv", [D], F32, kind="ExternalInput").ap()
    out = nc.dram_tensor("out", [N, D], CDT, kind="ExternalOutput").ap()
    with tile.TileContext(nc) as tc:
        _emit(tc, out, xT, wq, wk, wv, bqk, bv, repeat=repeat)
    nc.compile()
    return nc


def make_in_maps(inputs):
    x = np.asarray(inputs["x"], np.float32)
    bf = CDT_NP
    wq = np.asarray(inputs["Wq"], np.float32).astype(bf)
    # ec-major packing: wq2[ec, p, dc, c] = Wq[dc*128+p, ec*128+c]
    wq2 = np.ascontiguousarray(wq.reshape(DC, P, DC, P).transpose(2, 1, 0, 3))
    wk = np.asarray(inputs["Wk"], np.float32).astype(bf)
    wv = np.asarray(inputs["Wv"], np.float32).astype(bf)
    bq = np.asarray(inputs["bq"], np.float32)
    bk = np.asarray(inputs["bk"], np.float32)
    bv = np.ascontiguousarray(np.asarray(inputs["bv"], np.float32))
    # bqk[p, o] = bq[o*128+p]; bqk[p, DC+o] = bk[o*128+p]
    bqk = np.ascontiguousarray(
        np.concatenate([bq.reshape(DC, P).T, bk.reshape(DC, P).T], axis=1)
    )
    return [
        {
            "xT": np.ascontiguousarray(x[b].T).astype(bf),
            "wq": wq2,
            "wk": wk,
            "wv": wv,
            "bqk": bqk,
            "bv": bv,
        }
        for b in range(B)
    ]


_NC_CACHE = {}


def kernel(**inputs):
    global LAST_RESULT
    in_maps = make_in_maps(inputs)

    if 1 not in _NC_CACHE:
        _NC_CACHE[1] = _build()
    nc = _NC_CACHE[1]
    res = None
    for attempt in range(3):
        try:
            res = bass_utils.run_bass_kernel_spmd(nc, in_maps, core_ids=list(range(B)))
            break
        except Exception:
            if attempt == 2:
                raise
    LAST_RESULT = res
    return np.stack([res.results[c]["out"] for c in range(B)], axis=0).astype(np.float32)


if __name__ == "__main__":
    rng = np.random.default_rng(0)
    demo = {
        "x": rng.standard_normal((B, N, D), dtype=np.float32),
        "Wq": rng.uniform(-0.036, 0.036, (D, D)).astype(np.float32),
        "bq": rng.uniform(-0.036, 0.036, D).astype(np.float32),
        "Wk": rng.uniform(-0.036, 0.036, (D, D)).astype(np.float32),
        "bk": rng.uniform(-0.036, 0.036, D).astype(np.float32),
        "Wv": rng.uniform(-0.036, 0.036, (D, D)).astype(np.float32),
        "bv": rng.uniform(-0.036, 0.036, D).astype(np.float32),
    }
    out = kernel(**demo)
    print("out", out.shape, out.dtype, float(np.abs(out).max()))

